# revision 1
# baseline (speedup 1.0000x reference)
"""Trainium2 Bass kernel: patch-conv (Conv2d C3->E768, k4 s4) + giant linear.

y[b, eo] = sum_K flat[b, K] * wlin[eo, K] + blin[eo],
flat[b, e*256+p] = conv[b, e, p] (+ bconv[e]), K = 196608.

Strategy (8 cores, K-sharded over conv channel dim e, 96 channels/core):
  - Host: im2col of x -> xpT [49, B*256] (row 48 = ones, folds bconv into the
    conv matmul as a bias row). Pure index remap, zero FLOPs.
  - Each core: gets full xpT, its wconvT slice [49, 96] (row 48 = bconv slice)
    and its wlin column-slice [768, 24576] (contiguous K range).
    Device (bf16 compute, fp32 PSUM):
      conv: 512 matmuls (lhsT = xpT[:, b,p-half 128], rhs = wconvT [49,96])
            -> PSUM [128p, 96e] -> strided copy into flatT tiles
            T[ph] [128p, 96e, 256b] bf16.
      wlin: cast-DMA fp32->bf16 natural tiles [128eo, 2048K], PE-transpose
            [128,128] blocks -> PSUM -> copy into wlinT_kc [128K, 768eo] bf16.
      main: 192 K-chunks x 6 eo-chunks matmuls accumulating
            psum_y [128eo, 256b] fp32; write partial yT [768, 256] fp32.
  - Host: sum the 8 partials, transpose, add blin.
"""

import numpy as np

B, C, H, W = 256, 3, 64, 64
P, Hp, Wp, NP = 4, 16, 16, 256
E = 768
NCORES = 8
EL = E // NCORES          # 96 conv channels per core
KL = EL * NP              # 24576 contraction elems per core
KB = 2048                 # K columns per wlin DMA block
NKB = KL // KB            # 12 blocks
NKC = KL // 128           # 192 K-chunks
XQ = 16                   # conv b-batches (16 b's each)
BQ = B // XQ              # 16 b per batch

_CACHE = {}


def _build_bass():
    import concourse.bass as bass
    import concourse.mybir as mybir
    import concourse.tile as tile
    from concourse.masks import make_identity
    from contextlib import ExitStack

    dt = mybir.dt
    nc = bass.Bass()
    xpT_d = nc.dram_tensor("xpT", [49, B * NP], dt.float32, kind="ExternalInput")
    wcT_d = nc.dram_tensor("wconvT", [49, EL], dt.float32, kind="ExternalInput")
    wlin_d = nc.dram_tensor("wlin_s", [E, KL], dt.float32, kind="ExternalInput")
    out_d = nc.dram_tensor("yT", [E, B], dt.float32, kind="ExternalOutput")

    with tile.TileContext(nc) as tc, ExitStack() as ctx:
        singles = ctx.enter_context(tc.tile_pool(name="singles", bufs=1))
        identity = singles.tile([128, 128], dt.bfloat16)
        make_identity(nc, identity[:])
        wcT = singles.tile([49, EL], dt.bfloat16)
        nc.gpsimd.dma_start(out=wcT[:], in_=wcT_d[:])  # fp32 -> bf16 cast

        # Persistent flatT tiles: T[ph][p_row, e_local, b]. e-major so the main
        # matmul's moving operand (fixed e, all b) is contiguous — strided bf16
        # moving operands stream at ~1/8 rate (16B SBUF line granularity).
        tpool = ctx.enter_context(tc.tile_pool(name="flatT", bufs=1))
        T = [
            tpool.tile([128, EL, B], dt.bfloat16, tag=f"T{ph}", name=f"T{ph}")
            for ph in range(2)
        ]

        xp_pool = ctx.enter_context(tc.tile_pool(name="xq", bufs=2))
        wl_pool = ctx.enter_context(tc.tile_pool(name="wl_nat", bufs=12))
        wlt_pool = ctx.enter_context(tc.tile_pool(name="wlinT", bufs=6))
        out_pool = ctx.enter_context(tc.tile_pool(name="out_sb", bufs=1))

        # Warmup: absorb the identity (gpsimd) and wcT (DMA) readiness waits on
        # throwaway PE instructions so the first real matmul/transpose each
        # carry at most one semaphore wait (walrus allows one per Matmult).
        with tc.tile_pool(name="psum_w", bufs=1, space="PSUM") as pw:
            wps = pw.tile([128, 128], dt.bfloat16)
            nc.tensor.transpose(wps[:], identity[:], identity[:])
            wps2 = pw.tile([96, 49], dt.bfloat16, tag="w2")
            nc.tensor.transpose(wps2[:], wcT[:], identity[:49, :49])

        # ---- conv phase ----
        # 4 b's share one PSUM bank: the first matmul's start=True clears the
        # whole bank's has_written, siblings use start=False and so overwrite
        # their never-written regions. One copy per group then writes T with
        # an (e-outer, b-inner) AP -> 8-byte runs instead of 2-byte scatter.
        GB = 4  # b's per psum group
        xq_dmas = []
        with tc.tile_pool(name="psum_c", bufs=2, space="PSUM") as pc:
            for q in range(XQ):
                xq = xp_pool.tile([49, BQ * NP], dt.bfloat16)
                xq_dmas.append(
                    nc.gpsimd.dma_start(
                        out=xq[:], in_=xpT_d[:, q * BQ * NP : (q + 1) * BQ * NP]
                    )
                )
                for g in range(BQ // GB):
                    for ph in range(2):
                        pg = pc.tile([128, GB, EL], dt.float32)
                        for j in range(GB):
                            bl = g * GB + j
                            lhsT = xq[
                                :, bl * NP + ph * 128 : bl * NP + ph * 128 + 128
                            ]
                            nc.tensor.matmul(
                                pg[:, j, :],
                                lhsT,
                                wcT[:],
                                start=(j == 0),
                                stop=True,
                                skip_group_check=True,
                            )
                        b0 = q * BQ + g * GB
                        src = pg[:].rearrange("p j e -> p e j")
                        dst = T[ph][:, :, b0 : b0 + GB]
                        if (g + ph) % 2 == 0:
                            nc.vector.tensor_copy(dst, src)
                        else:
                            nc.scalar.copy(dst, src)

        # ---- wlin transpose + main matmul ----
        with (
            tc.tile_pool(name="psum_y", bufs=1, space="PSUM") as pyp,
            tc.tile_pool(name="psum_t", bufs=4, space="PSUM") as ptp,
        ):
            # Pack two 256-col accumulation regions per PSUM bank: at kc==0 the
            # even region's start=True clears the bank's has_written, the odd
            # region uses start=False and overwrites its never-written half.
            pys3 = [
                pyp.tile([128, 512], dt.float32, tag=f"py{i}", name=f"py{i}")
                for i in range(3)
            ]
            pys = [pys3[i // 2][:, (i % 2) * 256 : (i % 2) * 256 + 256] for i in range(6)]
            from concourse.tile import add_dep_helper

            for kb in range(NKB):
                wl_tiles = []
                for ec in range(6):
                    wt = wl_pool.tile([128, KB], dt.bfloat16)
                    dma = nc.gpsimd.dma_start(
                        out=wt[:],
                        in_=wlin_d[ec * 128 : (ec + 1) * 128, kb * KB : (kb + 1) * KB],
                    )
                    # Pace the wlin stream behind the conv's xq loads so the
                    # round-robin DMA queues don't starve the conv phase.
                    pace = min(kb + 1, XQ - 1)
                    add_dep_helper(
                        dma.ins, xq_dmas[pace].ins,
                        reason="pace wlin stream behind conv xq loads",
                    )
                    wl_tiles.append(wt)
                for kcl in range(KB // 128):
                    kc = kb * (KB // 128) + kcl
                    e_loc, ph = kc // 2, kc % 2
                    pst = ptp.tile([128, E], dt.bfloat16, tag="pt")
                    for ec in range(6):
                        src = wl_tiles[ec][:, kcl * 128 : (kcl + 1) * 128]
                        nc.tensor.transpose(
                            pst[:, ec * 128 : (ec + 1) * 128], src, identity[:]
                        )
                    wlt = wlt_pool.tile([128, E], dt.bfloat16)
                    nc.vector.tensor_copy(wlt[:, 0:384], pst[:, 0:384])
                    nc.scalar.copy(wlt[:, 384:768], pst[:, 384:768])
                    rhs = T[ph][:, e_loc, :]  # [128, 256] contiguous
                    for ec in range(6):
                        nc.tensor.matmul(
                            pys[ec][:],
                            wlt[:, ec * 128 : (ec + 1) * 128],
                            rhs,
                            start=(kc == 0 and ec % 2 == 0),
                            stop=(kc == NKC - 1),
                            skip_group_check=True,
                        )
            for i in range(3):
                ob = out_pool.tile([128, 512], dt.float32, tag=f"ob{i}")
                if i % 2 == 0:
                    nc.vector.tensor_copy(ob[:], pys3[i][:])
                else:
                    nc.scalar.copy(ob[:], pys3[i][:])
                nc.sync.dma_start(
                    out=out_d[2 * i * 128 : (2 * i + 1) * 128, :], in_=ob[:, 0:256]
                )
                nc.sync.dma_start(
                    out=out_d[(2 * i + 1) * 128 : (2 * i + 2) * 128, :],
                    in_=ob[:, 256:512],
                )
    _split_extra_waits(nc)
    return nc


def _split_extra_waits(nc):
    """Walrus encodes at most one semaphore wait on regular engine
    instructions (Matmult, DMACopy, ...). When Tile attaches more (e.g.
    slot-recycle release + data-ready on different procs), split the extras
    onto InstEventSemaphore instructions inserted immediately before the
    instruction on the same engine queue -- semantically identical to the
    multi-wait (the engine blocks at the same point for all of them)."""
    import bass_rust
    import concourse.mybir as mybir

    keep_multi = {"InstEventSemaphore", "InstUnconditionalBranch"}
    n_split = 0
    for fn in nc.m.functions:
        for bb in fn.blocks:
            out = []
            changed = False
            for ins in bb.instructions:
                si = ins.sync_info
                if (
                    si is not None
                    and len(si.on_wait) > 1
                    and type(ins).__name__ not in keep_multi
                ):
                    waits = list(si.on_wait)
                    for w in waits[:-1]:
                        ev = mybir.InstEventSemaphore(
                            name=f"W-split-{n_split}", ins=[], outs=[]
                        )
                        n_split += 1
                        ev.engine = ins.engine
                        ev.sync_info = bass_rust.SyncInfo(on_wait=[w], on_update=[])
                        out.append(ev)
                    ins.sync_info = bass_rust.SyncInfo(
                        on_wait=[waits[-1]], on_update=list(si.on_update)
                    )
                    changed = True
                out.append(ins)
            if changed:
                bb.instructions = out
    return n_split


def _prep_inputs(x, wconv, bconv, wlin):
    x = np.ascontiguousarray(np.asarray(x, dtype=np.float32))
    wconv = np.asarray(wconv, dtype=np.float32)
    bconv = np.asarray(bconv, dtype=np.float32)
    wlin = np.asarray(wlin, dtype=np.float32)

    # im2col: xpT[(c,i,j), (b, hp*16+wp)] = x[b, c, 4hp+i, 4wp+j]; row 48 = 1.
    xp = x.reshape(B, C, Hp, P, Wp, P).transpose(1, 3, 5, 0, 2, 4)
    xpT = np.empty((49, B * NP), np.float32)
    xpT[:48] = xp.reshape(48, B * NP)
    xpT[48] = 1.0

    wcT_full = wconv.reshape(E, 48).T  # [48, E]
    in_maps = []
    for k in range(NCORES):
        wcT_aug = np.empty((49, EL), np.float32)
        wcT_aug[:48] = wcT_full[:, k * EL : (k + 1) * EL]
        wcT_aug[48] = bconv[k * EL : (k + 1) * EL]
        wlin_s = np.ascontiguousarray(wlin[:, k * KL : (k + 1) * KL])
        in_maps.append({"xpT": xpT, "wconvT": wcT_aug, "wlin_s": wlin_s})
    return in_maps


def _patch_ldw_opt():
    """walrus is invoked with --enable-ldw-opt=false (hardcoded); enabling it
    lets codegen elide redundant LDWEIGHTS. Rewrite the flag on the way in."""
    from concourse import bass_utils as _bu

    if getattr(_bu, "_ldw_opt_patched", False):
        return
    _orig = _bu.run_command

    def _patched(cmd, **kw):
        if isinstance(cmd, list):
            cmd = [
                "--enable-ldw-opt=true" if c == "--enable-ldw-opt=false" else c
                for c in cmd
            ]
        return _orig(cmd, **kw)

    _bu.run_command = _patched
    _bu._ldw_opt_patched = True


def _run(x, wconv, bconv, wlin, blin, trace=False, **trace_kwargs):
    from concourse.bass_utils import run_bass_kernel_spmd


    if "nc" not in _CACHE:
        _CACHE["nc"] = _build_bass()
    in_maps = _prep_inputs(x, wconv, bconv, wlin)
    res = run_bass_kernel_spmd(
        _CACHE["nc"], in_maps, core_ids=list(range(NCORES)), trace=trace,
        **trace_kwargs,
    )
    acc = np.zeros((E, B), np.float64)
    for r in res.results:
        acc += r["yT"]
    y = (acc.T + np.asarray(blin, dtype=np.float64)[None, :]).astype(np.float32)
    return y, res


def kernel(x, wconv, bconv, wlin, blin, patch_size):
    assert int(patch_size) == P
    y, _ = _run(x, wconv, bconv, wlin, blin, trace=False)
    return y



# revision 6
# speedup vs baseline: 2.5635x; 2.5635x over previous
"""Trainium2 Bass kernel: patch-conv (Conv2d C3->E768, k4 s4) + giant linear.

y[b, eo] = sum_K flat[b, K] * wlin[eo, K] + blin[eo],
flat[b, e*256+p] = conv[b, e, p] (+ bconv[e]), K = 196608.

Reassociated algorithm (matrix-chain reordering, all FLOPs on device):
    y[b,eo] = sum_{p,cij} xp[cij,p,b] * W2[p,cij,eo]
    W2[p,cij,eo] = sum_e wca[cij,e] * wlin[eo, e*256+p]
where xp is the im2col of x (pure index remap, row 48 = ones) and wca is
wconv reshaped [49, E] (row 48 = bconv). This computes the identical
function with 19.7 GFLOP instead of 82 GFLOP, and reads wlin exactly once.

Sharding (8 cores): shard the patch index p (32 patches/core). Each core:
  - reads its wlin slice re-laid-out on host as wlp[e_r, (p, ech, eo)] bf16
    (37.75 MB -- the DMA roofline term),
  - W2-mm: for each p: 6 e-chunks x (N=512 + N=256) matmuls, stationary
    wce[e_chunk] = wcaT slice [128,49], accumulate PSUM [49,768],
  - copies W2_p to SBUF bf16 (vector/scalar alternating),
  - final-mm: stationary xpp[:, p, b-half] [49,128], moving W2_p, PSUM
    accumulates y-partial [128b, 512|256 eo] over the 32 p's.
  - Host: sum the 8 partial y's, add blin.
All PSUM tiles are bank-sized (2048B) so accumulation groups never cross
a PSUM bank: 4 banks y-accum + 2x2 banks W2-accum = 8.
"""

import numpy as np
import ml_dtypes

B, C, H, W = 256, 3, 64, 64
P, Hp, Wp, NP = 4, 16, 16, 256
E = 768
CIJ = 49                  # 48 conv taps + 1 bias row
NCORES = 8
PL = NP // NCORES         # 32 patches per core
NECH = E // 128           # 6 e-chunks in the W2 contraction
PCOL = NECH * E           # 4608 wlp columns per patch

_CACHE = {}


def _build_bass():
    import concourse.bass as bass
    import concourse.mybir as mybir
    import concourse.tile as tile
    from contextlib import ExitStack

    dt = mybir.dt
    nc = bass.Bass()
    wce_d = nc.dram_tensor("wce", [128, NECH * CIJ], dt.bfloat16, kind="ExternalInput")
    xpp_d = nc.dram_tensor("xpp", [CIJ, PL * B], dt.bfloat16, kind="ExternalInput")
    wlp_d = nc.dram_tensor("wlp", [128, PL * PCOL], dt.bfloat16, kind="ExternalInput")
    out_d = nc.dram_tensor("y", [B, E], dt.float32, kind="ExternalOutput")

    with tile.TileContext(nc) as tc, ExitStack() as ctx:
        singles = ctx.enter_context(tc.tile_pool(name="singles", bufs=1))
        wce = singles.tile([128, NECH * CIJ], dt.bfloat16)
        nc.gpsimd.dma_start(out=wce[:], in_=wce_d[:])
        xpp = singles.tile([CIJ, PL * B], dt.bfloat16)
        nc.sync.dma_start(out=xpp[:], in_=xpp_d[:])
        # W2 staging: [49, 32p * 768eo] bf16, written by V/S copies, read by
        # the final matmuls.
        w2sb = singles.tile([CIJ, PL * E], dt.bfloat16, tag="w2", name="w2sb")

        wlp_pool = ctx.enter_context(tc.tile_pool(name="wlp", bufs=4))
        out_pool = ctx.enter_context(tc.tile_pool(name="out_sb", bufs=1))

        # Warmup: absorb the wce DMA-ready wait on a throwaway matmul so the
        # first real W2 matmul carries only the wlp(0) wait.
        with tc.tile_pool(name="psum_w", bufs=1, space="PSUM") as pwarm:
            wm = pwarm.tile([CIJ, CIJ], dt.float32)
            nc.tensor.matmul(
                wm[:], wce[:, 0:CIJ], wce[:, 0:CIJ], start=True, stop=True,
                skip_group_check=True,
            )

        with (
            tc.tile_pool(name="psum_y", bufs=1, space="PSUM") as pyp,
            tc.tile_pool(name="psum_a", bufs=2, space="PSUM") as ppa,
            tc.tile_pool(name="psum_b", bufs=2, space="PSUM") as ppb,
        ):
            # y-partial accumulators: [128b x 512eo] + [128b x 256eo] per
            # b-half; each tile is a full PSUM bank.
            py = [
                [
                    pyp.tile([128, 512], dt.float32, tag=f"py{bh}0", name=f"py{bh}0"),
                    pyp.tile([128, 512], dt.float32, tag=f"py{bh}1", name=f"py{bh}1"),
                ]
                for bh in range(2)
            ]

            def w2_block(p):
                wl = wlp_pool.tile([128, PCOL], dt.bfloat16)
                nc.gpsimd.dma_start(
                    out=wl[:], in_=wlp_d[:, p * PCOL : (p + 1) * PCOL]
                )
                pa = ppa.tile([CIJ, 512], dt.float32)
                pb = ppb.tile([CIJ, 512], dt.float32)
                for ech in range(NECH):
                    lhsT = wce[:, ech * CIJ : (ech + 1) * CIJ]
                    nc.tensor.matmul(
                        pa[:, 0:512],
                        lhsT,
                        wl[:, ech * E : ech * E + 512],
                        start=(ech == 0),
                        stop=(ech == NECH - 1),
                        skip_group_check=True,
                    )
                    nc.tensor.matmul(
                        pb[:, 0:256],
                        lhsT,
                        wl[:, ech * E + 512 : ech * E + 768],
                        start=(ech == 0),
                        stop=(ech == NECH - 1),
                        skip_group_check=True,
                    )
                # PSUM f32 -> SBUF bf16 cast-copies; alternate engines by p.
                if p % 2 == 0:
                    nc.vector.tensor_copy(w2sb[:, p * E : p * E + 512], pa[:, 0:512])
                    nc.vector.tensor_copy(w2sb[:, p * E + 512 : p * E + 768], pb[:, 0:256])
                else:
                    nc.scalar.copy(w2sb[:, p * E : p * E + 512], pa[:, 0:512])
                    nc.scalar.copy(w2sb[:, p * E + 512 : p * E + 768], pb[:, 0:256])

            def final_block(p):
                for bh in range(2):
                    lhsT = xpp[:, p * B + bh * 128 : p * B + bh * 128 + 128]
                    nc.tensor.matmul(
                        py[bh][0][:, 0:512],
                        lhsT,
                        w2sb[:, p * E : p * E + 512],
                        start=(p == 0),
                        stop=(p == PL - 1),
                        skip_group_check=True,
                    )
                    nc.tensor.matmul(
                        py[bh][1][:, 0:256],
                        lhsT,
                        w2sb[:, p * E + 512 : p * E + 768],
                        start=(p == 0),
                        stop=(p == PL - 1),
                        skip_group_check=True,
                    )

            # Software pipeline: final(p) is emitted after W2(p+1) so the PE
            # never stalls on the PSUM->SBUF copy round-trip of W2(p).
            for p in range(PL):
                w2_block(p)
                if p >= 1:
                    final_block(p - 1)
            final_block(PL - 1)

            for bh in range(2):
                ob = out_pool.tile([128, E], dt.float32, tag=f"ob{bh}")
                nc.vector.tensor_copy(ob[:, 0:512], py[bh][0][:, 0:512])
                nc.scalar.copy(ob[:, 512:768], py[bh][1][:, 0:256])
                nc.sync.dma_start(
                    out=out_d[bh * 128 : (bh + 1) * 128, :], in_=ob[:]
                )
    _split_extra_waits(nc)
    return nc


def _split_extra_waits(nc):
    """Walrus encodes at most one semaphore wait on regular engine
    instructions (Matmult, DMACopy, ...). When Tile attaches more (e.g.
    slot-recycle release + data-ready on different procs), split the extras
    onto InstEventSemaphore instructions inserted immediately before the
    instruction on the same engine queue -- semantically identical to the
    multi-wait (the engine blocks at the same point for all of them)."""
    import bass_rust
    import concourse.mybir as mybir

    keep_multi = {"InstEventSemaphore", "InstUnconditionalBranch"}
    n_split = 0
    for fn in nc.m.functions:
        for bb in fn.blocks:
            out = []
            changed = False
            for ins in bb.instructions:
                si = ins.sync_info
                if (
                    si is not None
                    and len(si.on_wait) > 1
                    and type(ins).__name__ not in keep_multi
                ):
                    waits = list(si.on_wait)
                    for w in waits[:-1]:
                        ev = mybir.InstEventSemaphore(
                            name=f"W-split-{n_split}", ins=[], outs=[]
                        )
                        n_split += 1
                        ev.engine = ins.engine
                        ev.sync_info = bass_rust.SyncInfo(on_wait=[w], on_update=[])
                        out.append(ev)
                    ins.sync_info = bass_rust.SyncInfo(
                        on_wait=[waits[-1]], on_update=list(si.on_update)
                    )
                    changed = True
                out.append(ins)
            if changed:
                bb.instructions = out
    return n_split


def _prep_inputs(x, wconv, bconv, wlin):
    bf16 = ml_dtypes.bfloat16
    x = np.ascontiguousarray(np.asarray(x, dtype=np.float32))
    wconv = np.asarray(wconv, dtype=np.float32)
    bconv = np.asarray(bconv, dtype=np.float32)
    wlin = np.asarray(wlin, dtype=np.float32)

    # im2col: xpa[(c,i,j), b, p] = x[b, c, 4hp+i, 4wp+j], p = hp*16+wp;
    # row 48 = ones (bias row). Pure index remap, zero FLOPs.
    xp = x.reshape(B, C, Hp, P, Wp, P).transpose(1, 3, 5, 0, 2, 4)
    xpa = np.empty((CIJ, B, NP), np.float32)
    xpa[:48] = xp.reshape(48, B, NP)
    xpa[48] = 1.0

    # wce[e_r, ech, cij] = wcaT[ech*128+e_r, cij]; wca row 48 = bconv.
    wca = np.empty((CIJ, E), np.float32)
    wca[:48] = wconv.reshape(E, 48).T
    wca[48] = bconv
    wce = np.ascontiguousarray(
        wca.T.reshape(NECH, 128, CIJ).transpose(1, 0, 2).reshape(128, NECH * CIJ)
    ).astype(bf16)

    wlinR = wlin.reshape(E, E, NP)  # [eo, e, p]
    in_maps = []
    for c in range(NCORES):
        ps = c * PL
        # wlp[e_r, p*4608 + ech*768 + eo] = wlin[eo, (ech*128+e_r)*256 + p]
        wlp = (
            wlinR[:, :, ps : ps + PL]
            .transpose(1, 2, 0)                 # [e, p, eo]
            .reshape(NECH, 128, PL, E)
            .transpose(1, 2, 0, 3)              # [e_r, p, ech, eo]
            .reshape(128, PL * PCOL)
            .astype(bf16)
        )
        xpp = (
            xpa[:, :, ps : ps + PL]
            .transpose(0, 2, 1)                 # [cij, p, b]
            .reshape(CIJ, PL * B)
            .astype(bf16)
        )
        in_maps.append({"wce": wce, "xpp": xpp, "wlp": wlp})
    return in_maps


def _patch_ldw_opt():
    """walrus is invoked with --enable-ldw-opt=false (hardcoded); enabling it
    lets codegen elide redundant LDWEIGHTS. Rewrite the flag on the way in."""
    from concourse import bass_utils as _bu

    if getattr(_bu, "_ldw_opt_patched", False):
        return
    _orig = _bu.run_command

    def _patched(cmd, **kw):
        if isinstance(cmd, list):
            cmd = [
                "--enable-ldw-opt=true" if c == "--enable-ldw-opt=false" else c
                for c in cmd
            ]
        return _orig(cmd, **kw)

    _bu.run_command = _patched
    _bu._ldw_opt_patched = True


def _run(x, wconv, bconv, wlin, blin, trace=False, **trace_kwargs):
    from concourse.bass_utils import run_bass_kernel_spmd

    if "nc" not in _CACHE:
        _CACHE["nc"] = _build_bass()
    in_maps = _prep_inputs(x, wconv, bconv, wlin)
    res = run_bass_kernel_spmd(
        _CACHE["nc"], in_maps, core_ids=list(range(NCORES)), trace=trace,
        **trace_kwargs,
    )
    acc = np.zeros((B, E), np.float64)
    for r in res.results:
        acc += r["y"]
    y = (acc + np.asarray(blin, dtype=np.float64)[None, :]).astype(np.float32)
    return y, res


def kernel(x, wconv, bconv, wlin, blin, patch_size):
    assert int(patch_size) == P
    y, _ = _run(x, wconv, bconv, wlin, blin, trace=False)
    return y
